# revision 4
# baseline (speedup 1.0000x reference)
"""DBRX MoE experts kernel for Trainium2 (8 NeuronCores, tensor-parallel).

Strategy (tensor-parallel over the intermediate dim I):
  - Host: router (softmax top-2 + renorm), tokens dispatched into one stream
    ordered by expert (exact per-expert counts baked into the compiled
    program), weights sliced along I into 8 shards.
  - Device core c (SPMD, identical program, different weight shards):
    for each expert segment: h = silu(X @ w1_c.T) * (X @ v1_c.T) over its
    I/8 slice, then y_c = h @ w2_c.T — a PARTIAL sum of the output.
    bf16 matmuls, fp32 PSUM; every matmul streams >=330 tokens so the next
    stationary-weight load hides under it.
  - Host: y = sum_c y_c, then combine with gates.

Perfect load balance (every core does 1/8 of every token-expert pair), vs
expert-parallel where all cores pad to the busiest expert's token count.
"""

import numpy as np

import concourse.bass as bass
from concourse import bacc, mybir, tile
from concourse.bass_utils import run_bass_kernel_spmd

BF16 = mybir.dt.bfloat16
F32 = mybir.dt.float32
NP_BF16 = mybir.dt.np(BF16)

P = 128
NCHUNK = 512


def _c_chunks(C):
    """Equal-ish chunks <= 512 so every matmul is wide enough to hide the
    next LDWEIGHTS."""
    k = (C + NCHUNK - 1) // NCHUNK
    base, rem = divmod(C, k)
    out, o = [], 0
    for j in range(k):
        s = base + (1 if j < rem else 0)
        out.append((o, s))
        o += s
    return out


def build_nc_tp(counts, D, I, E=8, num_devices=8, iters=1, y_bf16=True,
                wload_bufs=4, xload_bufs=4, split_w=False, y_ring='sync'):
    nD = D // P
    I8 = I // num_devices
    nI8 = I8 // P
    S = int(sum(counts))
    offs = np.concatenate([[0], np.cumsum(counts)]).astype(int)
    YDT = BF16 if y_bf16 else F32

    nc = bacc.Bacc(
        "TRN2", target_bir_lowering=False, debug=False, num_devices=num_devices
    )
    xt_d = nc.dram_tensor("xt", [P, nD * S], BF16, kind="ExternalInput").ap()
    w1_d = nc.dram_tensor("w1t", [E * nI8, P, D], BF16, kind="ExternalInput").ap()
    v1_d = nc.dram_tensor("v1t", [E * nI8, P, D], BF16, kind="ExternalInput").ap()
    w2_d = nc.dram_tensor("w2t", [E, P, nD * I8], BF16, kind="ExternalInput").ap()
    y_d = nc.dram_tensor("y", [nD, P, S], YDT, kind="ExternalOutput").ap()

    with tile.TileContext(nc) as tc:
        with (
            tc.tile_pool(name="xload", bufs=xload_bufs) as xload,
            tc.tile_pool(name="h2res", bufs=2) as h2res,
            tc.tile_pool(name="wload", bufs=wload_bufs) as wload,
            tc.tile_pool(name="w2load", bufs=2) as w2load,
            tc.tile_pool(name="yout", bufs=12) as yout,
            tc.tile_pool(name="sgp", bufs=4) as sgp,
            tc.tile_pool(name="ps", bufs=8, space="PSUM") as ps,
        ):
          for _rep in range(iters):
            for e in range(E):
                Ce = int(counts[e])
                off0 = int(offs[e])
                chunks = _c_chunks(Ce)

                # x chunks: 2 DMAs each on gpsimd (cheap SWDGE trigger count,
                # halves head-of-line blocking on the DMA fabric)
                xcs = []
                for j, (co, cs) in enumerate(chunks):
                    xc = xload.tile([P, nD, cs], BF16, tag="xc")
                    g = nD * (off0 + co)
                    h = nD // 2
                    nc.gpsimd.dma_start(xc[:, :h], xt_d[:, g : g + h * cs])
                    nc.gpsimd.dma_start(
                        xc[:, h:], xt_d[:, g + h * cs : g + nD * cs]
                    )
                    xcs.append(xc)
                w2sb = w2load.tile([P, nD, I8], BF16, tag="w2")
                nc.gpsimd.dma_start(w2sb[:, : nD // 2], w2_d[e][:, : nD * I8 // 2])
                nc.gpsimd.dma_start(w2sb[:, nD // 2 :], w2_d[e][:, nD * I8 // 2 :])

                # up-projection weight shard (sync/HWDGE queue, with y-out)
                if split_w:
                    w1ts, v1ts = [], []
                    for ii in range(nI8):
                        w1t_ii = wload.tile([P, nD, P], BF16, tag="wi",
                                            name="w1t_ii")
                        v1t_ii = wload.tile([P, nD, P], BF16, tag="wi",
                                            name="v1t_ii")
                        nc.sync.dma_start(w1t_ii, w1_d[e * nI8 + ii])
                        nc.sync.dma_start(v1t_ii, v1_d[e * nI8 + ii])
                        w1ts.append(w1t_ii)
                        v1ts.append(v1t_ii)
                    class _W:
                        def __init__(self, ts):
                            self.ts = ts
                        def __getitem__(self, key):
                            _, ii, d, _ = key
                            return self.ts[ii][:, d, :]
                    w1sb = _W(w1ts)
                    v1sb = _W(v1ts)
                else:
                    w1sb = wload.tile([P, nI8, nD, P], BF16, tag="w")
                    v1sb = wload.tile([P, nI8, nD, P], BF16, tag="w")
                    for ii in range(nI8):
                        nc.sync.dma_start(w1sb[:, ii], w1_d[e * nI8 + ii])
                        nc.sync.dma_start(v1sb[:, ii], v1_d[e * nI8 + ii])

                h2_sb = h2res.tile([P, nI8, Ce], BF16, tag="h2")

                # Phase 1: chunk-OUTER — 2 live PSUM banks per step, so the
                # silu release staggers and the next iteration's matmuls never
                # wait on banks; phase2 of early chunks overlaps later chunks
                for j, (co, cs) in enumerate(chunks):
                    for ii in range(nI8):
                        ph = ps.tile([P, cs], F32, tag="pp", name="ph")
                        pg = ps.tile([P, cs], F32, tag="pp", name="pg")
                        for d in range(nD):
                            nc.tensor.matmul(
                                ph[:, :cs],
                                w1sb[:, ii, d, :],
                                xcs[j][:, d, :],
                                start=(d == 0),
                                stop=(d == nD - 1),
                            )
                        for d in range(nD):
                            nc.tensor.matmul(
                                pg[:, :cs],
                                v1sb[:, ii, d, :],
                                xcs[j][:, d, :],
                                start=(d == 0),
                                stop=(d == nD - 1),
                            )
                        t1 = sgp.tile([P, NCHUNK], F32, tag="t1")
                        nc.scalar.activation(
                            t1[:, :cs], ph[:, :cs],
                            mybir.ActivationFunctionType.Silu,
                        )
                        nc.vector.tensor_mul(
                            h2_sb[:, ii, co : co + cs], t1[:, :cs], pg[:, :cs]
                        )

                # Phase 2: py[dt, chunk] = sum_ii w2[dt,ii] @ h2[ii, chunk]
                for dt in range(nD):
                    py = [
                        ps.tile([P, cs], F32, tag="pp", name="py")
                        for _, cs in chunks
                    ]
                    for ic in range(nI8):
                        for j, (co, cs) in enumerate(chunks):
                            nc.tensor.matmul(
                                py[j][:, :cs],
                                w2sb[:, dt, 128 * ic : 128 * (ic + 1)],
                                h2_sb[:, ic, co : co + cs],
                                start=(ic == 0),
                                stop=(ic == nI8 - 1),
                            )
                    for j, (co, cs) in enumerate(chunks):
                        ysb = yout.tile([P, NCHUNK], YDT, tag="y")
                        if (dt * len(chunks) + j) % 2 == 0:
                            nc.vector.tensor_copy(ysb[:, :cs], py[j][:, :cs])
                        else:
                            nc.scalar.activation(
                                ysb[:, :cs], py[j][:, :cs],
                                mybir.ActivationFunctionType.Copy,
                            )
                        y_eng = nc.sync if y_ring == 'sync' else nc.scalar
                        y_eng.dma_start(
                            y_d[dt][:, off0 + co : off0 + co + cs], ysb[:, :cs]
                        )

    nc.compile()
    return nc


def pack_w_up(w):
    """[I8, D] -> [I8//P, 128, D] lhsT tiles."""
    I, D = w.shape
    a = w.reshape(I // P, P, D // P, P)
    return np.ascontiguousarray(
        a.transpose(0, 3, 2, 1).reshape(I // P, P, D)
    ).astype(NP_BF16)


def pack_w_down(w):
    """[D, I8] -> [D//P, 128, I8] lhsT tiles."""
    D, I = w.shape
    a = w.reshape(D // P, P, I // P, P)
    return np.ascontiguousarray(
        a.transpose(0, 3, 2, 1).reshape(D // P, P, I)
    ).astype(NP_BF16)


def unpack_y(y, S):
    """[nD, 128, S] -> [S, D]."""
    return y.transpose(2, 0, 1).reshape(S, -1)


def route(x, wr, top_k=2):
    logits = x @ wr.T
    logits -= logits.max(-1, keepdims=True)
    p = np.exp(logits, dtype=np.float32)
    p /= p.sum(-1, keepdims=True)
    topi = np.argpartition(-p, top_k - 1, axis=-1)[:, :top_k]
    topw = np.take_along_axis(p, topi, -1)
    topw = topw / topw.sum(-1, keepdims=True)
    return topi, topw


def tp_prepare(x, w1, v1, w2, topi, num_devices=8):
    T, D = x.shape
    E, I, _ = w1.shape
    I8 = I // num_devices
    nD = D // P
    idx = [np.nonzero((topi == e).any(-1))[0] for e in range(E)]
    counts = [max(8, ((len(ix) + 7) // 8) * 8) for ix in idx]
    S = sum(counts)
    offs = np.concatenate([[0], np.cumsum(counts)]).astype(int)

    xd = np.zeros((S, D), np.float32)
    for e in range(E):
        xd[offs[e] : offs[e] + len(idx[e])] = x[idx[e]]
    xb = np.ascontiguousarray(xd.reshape(S, nD, P).transpose(2, 1, 0)).astype(
        NP_BF16
    )
    blocks = []
    for e in range(E):
        for co, cs in _c_chunks(int(counts[e])):
            g = offs[e] + co
            blocks.append(xb[:, :, g : g + cs].reshape(P, nD * cs))
    xt = np.ascontiguousarray(np.concatenate(blocks, axis=1))

    in_maps = []
    for c in range(num_devices):
        sl = slice(c * I8, (c + 1) * I8)
        w1t = np.concatenate([pack_w_up(w1[e][sl]) for e in range(E)], axis=0)
        v1t = np.concatenate([pack_w_up(v1[e][sl]) for e in range(E)], axis=0)
        w2t = np.stack(
            [
                pack_w_down(w2[e][:, sl]).transpose(1, 0, 2).reshape(P, nD * I8)
                for e in range(E)
            ],
            axis=0,
        )
        in_maps.append({"xt": xt, "w1t": w1t, "v1t": v1t, "w2t": w2t})
    return in_maps, counts, offs, idx


_NC_CACHE = {}


def kernel(hidden_states, wr, w1, v1, w2, index):
    x = np.asarray(hidden_states, dtype=np.float32)
    wr = np.asarray(wr, dtype=np.float32)
    w1 = np.asarray(w1, dtype=np.float32)
    v1 = np.asarray(v1, dtype=np.float32)
    w2 = np.asarray(w2, dtype=np.float32)
    T, D = x.shape
    E, I, _ = w1.shape

    topi, topw = route(x, wr)
    gates = np.zeros((T, E), np.float32)
    np.put_along_axis(gates, topi, topw, axis=-1)

    in_maps, counts, offs, idx = tp_prepare(x, w1, v1, w2, topi, E)
    key = (tuple(counts), D, I, E)
    if key not in _NC_CACHE:
        _NC_CACHE[key] = build_nc_tp(tuple(counts), D, I, E=E, num_devices=E)
    nc = _NC_CACHE[key]

    res = run_bass_kernel_spmd(nc, in_maps, core_ids=list(range(E)))

    S = sum(counts)
    ysum = np.zeros((S, D), np.float32)
    for c in range(E):
        ysum += unpack_y(res.results[c]["y"].astype(np.float32), S)
    out = np.zeros((T, D), np.float32)
    for e in range(E):
        n = len(idx[e])
        out[idx[e]] += gates[idx[e], e][:, None] * ysum[offs[e] : offs[e] + n]
    return out

